# revision 2
# baseline (speedup 1.0000x reference)
"""MoE action layer (4 experts, top-2) on 8 Trainium2 NeuronCores.

Strategy (expert-parallel, intermediate-dim split):
- Host: router matmul + softmax + top-2 + gate renormalization (0.03% of
  FLOPs), gather tokens per expert, pack/cast operands to bf16.
- Device: 8 cores, SPMD. Core c handles expert e=c//2, intermediate-dim half
  h=c%2. It computes, for all tokens routed to e (padded to capacity C):
      Y_c = gate * (GELU(X_e @ W1[e][:, half] + b1[e][half]) @ W2[e][half, :])
  Both matmuls in bf16 with fp32 PSUM accumulation; GELU is the exact-erf
  ACT LUT. The pair's partial Y's are summed on host (f-split of the MLP),
  then scatter-added into the final output (each token hits 2 experts).
- b2 contribution (gate-weighted, sums to b2 since gates sum to 1 per token)
  is added on host: out += gate_matrix @ b2.

Everything here is self-contained; shapes are taken from the inputs.
"""

import os

import numpy as np
import ml_dtypes

import concourse.mybir as mybir
import concourse.tile as tile
from concourse import bacc
from concourse.bass_utils import run_bass_kernel_spmd

BF16 = ml_dtypes.bfloat16
N_CORES = 8
TOP_K = 2
O_TILES = 4          # output-column tiles
CHUNK = 512          # token chunk (moving free dim for matmul 1)

_nc_cache = {}       # (C, H, FH, O) -> (nc, out_name)
_pack_cache = {}     # id-based cache of packed weights

# exposed for test harness
last_results = None


def _build(C, H, FH, O, repeat=1):
    """Build + compile the SPMD Tile kernel for capacity C tokens."""
    key = (C, H, FH, O, repeat)
    if key in _nc_cache:
        return _nc_cache[key]

    KH = H // 128            # contraction tiles for matmul 1
    KF = FH // 128           # contraction tiles for matmul 2
    assert O % O_TILES == 0
    OT = O // O_TILES        # 448 for O=1792
    n_t = C // 128           # total 128-token tiles
    chunks = []
    rem = C
    while rem > 0:
        t = min(CHUNK, rem)
        chunks.append(t)
        rem -= t

    dt = mybir.dt
    nc = bacc.Bacc("TRN2", target_bir_lowering=False, debug=False,
                   num_devices=N_CORES)

    xt_d = nc.dram_tensor("xt", [128, KH, C], dt.bfloat16, kind="ExternalInput")
    w1_d = nc.dram_tensor("w1", [KF, 128, KH, 128], dt.bfloat16, kind="ExternalInput")
    w2_d = nc.dram_tensor("w2", [O_TILES, 128, KF, OT], dt.bfloat16, kind="ExternalInput")
    b1_d = nc.dram_tensor("b1p", [128, KF], dt.float32, kind="ExternalInput")
    g_d = nc.dram_tensor("gatep", [128, n_t], dt.float32, kind="ExternalInput")
    out_d = nc.dram_tensor("out", [C, O], dt.float32, kind="ExternalOutput")

    with tile.TileContext(nc) as tc:
        with (
            tc.tile_pool(name="xp", bufs=1) as xpool,
            tc.tile_pool(name="hp", bufs=1) as hpool,
            tc.tile_pool(name="w1p", bufs=3) as w1pool,
            tc.tile_pool(name="w2p", bufs=2) as w2pool,
            tc.tile_pool(name="op", bufs=4) as opool,
            tc.tile_pool(name="cp", bufs=1) as cpool,
            tc.tile_pool(name="ps1", bufs=3, space="PSUM") as psum1,
            tc.tile_pool(name="ps2", bufs=3, space="PSUM") as psum2,
        ):
            b1_s = cpool.tile([128, KF], dt.float32)
            g_s = cpool.tile([128, n_t], dt.float32)
            nc.sync.dma_start(b1_s[:], b1_d[:])
            nc.sync.dma_start(g_s[:], g_d[:])

            def body():
                t0 = 0
                for T in chunks:
                    xt_s = xpool.tile([128, KH, T], dt.bfloat16, tag="xt")
                    nc.sync.dma_start(xt_s[:], xt_d[:, :, t0:t0 + T])
                    hT_s = hpool.tile([128, KF, T], dt.bfloat16, tag="ht")
                    # ---- phase 1: hT[f, t] = gelu(W1c.T @ X + b1) ----
                    for ft in range(KF):
                        w1_s = w1pool.tile([128, KH, 128], dt.bfloat16, tag="w1")
                        nc.sync.dma_start(w1_s[:], w1_d[ft])
                        ps = psum1.tile([128, T], dt.float32, tag="ps1")
                        for hk in range(KH):
                            nc.tensor.matmul(
                                ps[:], w1_s[:, hk, :], xt_s[:, hk, :],
                                start=(hk == 0), stop=(hk == KH - 1),
                            )
                        nc.scalar.activation(
                            hT_s[:, ft, :], ps[:],
                            mybir.ActivationFunctionType.Gelu,
                            bias=b1_s[:, ft:ft + 1],
                        )
                    # ---- phase 2: Y[t, o] = gate * (hT.T @ W2c) ----
                    for ot in range(O_TILES):
                        w2_s = w2pool.tile([128, KF, OT], dt.bfloat16, tag="w2")
                        nc.sync.dma_start(w2_s[:], w2_d[ot])
                        for tt in range(T // 128):
                            ps2 = psum2.tile([128, OT], dt.float32, tag="ps2")
                            for fk in range(KF):
                                nc.tensor.matmul(
                                    ps2[:],
                                    hT_s[:, fk, tt * 128:(tt + 1) * 128],
                                    w2_s[:, fk, :],
                                    start=(fk == 0), stop=(fk == KF - 1),
                                )
                            o_s = opool.tile([128, OT], dt.float32, tag="o")
                            gi = t0 // 128 + tt
                            nc.vector.tensor_scalar_mul(
                                o_s[:], ps2[:], g_s[:, gi:gi + 1])
                            nc.sync.dma_start(
                                out_d[t0 + tt * 128: t0 + (tt + 1) * 128,
                                      ot * OT:(ot + 1) * OT],
                                o_s[:])
                    t0 += T

            if repeat > 1:
                with tc.For_i(0, repeat, 1):
                    body()
            else:
                body()

    nc.compile()
    _nc_cache[key] = (nc, out_d.name)
    return nc, out_d.name


def _pack_weights(w1, b1, w2, H, FH, O):
    """Pack per-core weight operands (cached on input array identity)."""
    key = (id(w1), id(w2), id(b1))
    if key in _pack_cache:
        return _pack_cache[key]
    KH = H // 128
    KF = FH // 128
    OT = O // O_TILES
    E = w1.shape[0]
    packs = []
    for e in range(E):
        w1e = np.asarray(w1[e]).astype(BF16)      # [H, 2FH]
        w2e = np.asarray(w2[e]).astype(BF16)      # [2FH, O]
        b1e = np.asarray(b1[e], dtype=np.float32)  # [2FH]
        for half in range(2):
            w1c = w1e[:, half * FH:(half + 1) * FH]
            w2c = w2e[half * FH:(half + 1) * FH, :]
            b1c = b1e[half * FH:(half + 1) * FH]
            w1p = np.ascontiguousarray(
                w1c.reshape(KH, 128, KF, 128).transpose(2, 1, 0, 3))
            w2p = np.ascontiguousarray(
                w2c.reshape(KF, 128, O_TILES, OT).transpose(2, 1, 0, 3))
            b1p = np.ascontiguousarray(b1c.reshape(KF, 128).T)
            packs.append((w1p, w2p, b1p))
    _pack_cache.clear()
    _pack_cache[key] = packs
    return packs


def kernel(hidden_states, router_w, router_b, w1, b1, w2, b2):
    global last_results
    hs = np.asarray(hidden_states, dtype=np.float32)
    rw = np.asarray(router_w, dtype=np.float32)
    rb = np.asarray(router_b, dtype=np.float32)
    w1 = np.asarray(w1)
    w2 = np.asarray(w2)
    b1 = np.asarray(b1)
    b2 = np.asarray(b2, dtype=np.float32)

    B, H = hs.shape
    E = rw.shape[1]
    F2 = w1.shape[2]
    FH = F2 // 2
    O = w2.shape[2]
    assert E * 2 == N_CORES

    # ---- host routing ----
    logits = hs @ rw + rb                       # [B, E] fp32
    m = logits.max(axis=-1, keepdims=True)
    ex = np.exp(logits - m)
    probs = ex / ex.sum(axis=-1, keepdims=True)
    idx = np.argsort(-probs, axis=-1, kind="stable")[:, :TOP_K]   # [B, K]
    p_top = np.take_along_axis(probs, idx, axis=-1)
    gates = (p_top / p_top.sum(axis=-1, keepdims=True)).astype(np.float32)
    gmat = np.zeros((B, E), np.float32)
    np.put_along_axis(gmat, idx, gates, axis=-1)

    sel = np.zeros((B, E), bool)
    np.put_along_axis(sel, idx, True, axis=-1)
    tok = [np.nonzero(sel[:, e])[0] for e in range(E)]
    counts = [len(t) for t in tok]

    # capacity (multiple of 128, same for all cores — SPMD single program)
    C = max(256, int(-(-max(counts) // 128)) * 128)
    KH = H // 128
    n_t = C // 128

    repeat = int(os.environ.get("BASS_KERNEL_REPEAT", "1"))
    nc, out_name = _build(C, H, FH, O, repeat=repeat)
    packs = _pack_weights(w1, b1, w2, H, FH, O)

    # ---- per-core inputs ----
    in_maps = []
    xts = {}
    gps = {}
    for e in range(E):
        n_e = counts[e]
        Xe = np.zeros((C, H), np.float32)
        Xe[:n_e] = hs[tok[e]]
        xt = np.ascontiguousarray(
            Xe.T.astype(BF16).reshape(KH, 128, C).transpose(1, 0, 2))
        xts[e] = xt
        gp = np.zeros((C,), np.float32)
        gp[:n_e] = gmat[tok[e], e]
        gps[e] = np.ascontiguousarray(gp.reshape(n_t, 128).T)
    for c in range(N_CORES):
        e, half = c // 2, c % 2
        w1p, w2p, b1p = packs[c]
        in_maps.append({
            "xt": xts[e], "w1": w1p, "w2": w2p, "b1p": b1p, "gatep": gps[e],
        })

    res = run_bass_kernel_spmd(nc, in_maps, core_ids=list(range(N_CORES)))
    last_results = res

    # ---- host combine ----
    out = gmat @ b2.astype(np.float32)           # [B, O]; zeros in practice
    for e in range(E):
        Y = res.results[2 * e][out_name] + res.results[2 * e + 1][out_name]
        out[tok[e]] += Y[:counts[e]]

    if O == 1792:  # ACTION_DIM=7, NUM_BINS=256
        return out.reshape(B, 7, 256)
    return out.reshape(B, O)


# revision 3
# speedup vs baseline: 4.0991x; 4.0991x over previous
"""MoE action layer (4 experts, top-2) on 8 Trainium2 NeuronCores.

Strategy (expert-parallel, intermediate-dim split):
- Host: router matmul + softmax + top-2 + gate renormalization (0.03% of
  FLOPs), gather tokens per expert, pack/cast operands to bf16.
- Device: 8 cores, SPMD. Core c handles expert e=c//2, intermediate-dim half
  h=c%2. It computes, for all tokens routed to e (padded to capacity C):
      Y_c = gate * (GELU(X_e @ W1[e][:, half] + b1[e][half]) @ W2[e][half, :])
  Both matmuls in bf16 with fp32 PSUM accumulation; GELU is the exact-erf
  ACT LUT. The pair's partial Y's are summed on host (f-split of the MLP),
  then scatter-added into the final output (each token hits 2 experts).
- b2 contribution (gate-weighted, sums to b2 since gates sum to 1 per token)
  is added on host: out += gate_matrix @ b2.

Everything here is self-contained; shapes are taken from the inputs.
"""

import os

import numpy as np
import ml_dtypes

import concourse.mybir as mybir
import concourse.tile as tile
from concourse import bacc
from concourse.bass_utils import run_bass_kernel_spmd

BF16 = ml_dtypes.bfloat16
N_CORES = 8
TOP_K = 2
O_TILES = 4          # output-column tiles
CHUNK = 512          # token chunk (moving free dim for matmul 1)

_nc_cache = {}       # (C, H, FH, O) -> (nc, out_name)
_pack_cache = {}     # id-based cache of packed weights

# exposed for test harness
last_results = None


def _build(C, H, FH, O, repeat=1):
    """Build + compile the SPMD Tile kernel for capacity C tokens."""
    key = (C, H, FH, O, repeat)
    if key in _nc_cache:
        return _nc_cache[key]

    KH = H // 128            # contraction tiles for matmul 1
    KF = FH // 128           # contraction tiles for matmul 2
    assert O % O_TILES == 0
    OT = O // O_TILES        # 448 for O=1792
    n_t = C // 128           # total 128-token tiles
    chunks = []
    rem = C
    while rem > 0:
        t = min(CHUNK, rem)
        chunks.append(t)
        rem -= t

    dt = mybir.dt
    nc = bacc.Bacc("TRN2", target_bir_lowering=False, debug=False,
                   num_devices=N_CORES)

    xt_d = nc.dram_tensor("xt", [128, KH, C], dt.bfloat16, kind="ExternalInput")
    w1_d = nc.dram_tensor("w1", [KF, 128, KH, 128], dt.bfloat16, kind="ExternalInput")
    w2_d = nc.dram_tensor("w2", [O_TILES, 128, KF, OT], dt.bfloat16, kind="ExternalInput")
    b1_d = nc.dram_tensor("b1p", [128, KF], dt.float32, kind="ExternalInput")
    g_d = nc.dram_tensor("gatep", [128, n_t], dt.float32, kind="ExternalInput")
    out_d = nc.dram_tensor("out", [C, O], dt.float32, kind="ExternalOutput")

    with tile.TileContext(nc) as tc:
        with (
            tc.tile_pool(name="xp", bufs=1) as xpool,
            tc.tile_pool(name="hp", bufs=2) as hpool,
            tc.tile_pool(name="w1p", bufs=3) as w1pool,
            tc.tile_pool(name="w2p", bufs=2) as w2pool,
            tc.tile_pool(name="op", bufs=4) as opool,
            tc.tile_pool(name="cp", bufs=1) as cpool,
            tc.tile_pool(name="ps1", bufs=3, space="PSUM") as psum1,
            tc.tile_pool(name="ps2", bufs=3, space="PSUM") as psum2,
        ):
            b1_s = cpool.tile([128, KF], dt.float32)
            g_s = cpool.tile([128, n_t], dt.float32)
            nc.sync.dma_start(b1_s[:], b1_d[:])
            nc.sync.dma_start(g_s[:], g_d[:])

            def body():
                t0 = 0
                for T in chunks:
                    xt_s = xpool.tile([128, KH, T], dt.bfloat16, tag="xt")
                    nc.sync.dma_start(xt_s[:], xt_d[:, :, t0:t0 + T])
                    hT_s = hpool.tile([128, KF, T], dt.bfloat16, tag="ht")
                    # ---- phase 1: hT[f, t] = gelu(W1c.T @ X + b1) ----
                    for ft in range(KF):
                        w1_s = w1pool.tile([128, KH, 128], dt.bfloat16, tag="w1")
                        nc.sync.dma_start(w1_s[:], w1_d[ft])
                        ps = psum1.tile([128, T], dt.float32, tag="ps1")
                        for hk in range(KH):
                            nc.tensor.matmul(
                                ps[:], w1_s[:, hk, :], xt_s[:, hk, :],
                                start=(hk == 0), stop=(hk == KH - 1),
                            )
                        nc.scalar.activation(
                            hT_s[:, ft, :], ps[:],
                            mybir.ActivationFunctionType.Gelu,
                            bias=b1_s[:, ft:ft + 1],
                        )
                    # ---- phase 2: Y[t, o] = gate * (hT.T @ W2c) ----
                    for ot in range(O_TILES):
                        w2_s = w2pool.tile([128, KF, OT], dt.bfloat16, tag="w2")
                        nc.sync.dma_start(w2_s[:], w2_d[ot])
                        for tt in range(T // 128):
                            ps2 = psum2.tile([128, OT], dt.float32, tag="ps2")
                            for fk in range(KF):
                                nc.tensor.matmul(
                                    ps2[:],
                                    hT_s[:, fk, tt * 128:(tt + 1) * 128],
                                    w2_s[:, fk, :],
                                    start=(fk == 0), stop=(fk == KF - 1),
                                )
                            o_s = opool.tile([128, OT], dt.float32, tag="o")
                            gi = t0 // 128 + tt
                            nc.vector.tensor_scalar_mul(
                                o_s[:], ps2[:], g_s[:, gi:gi + 1])
                            nc.sync.dma_start(
                                out_d[t0 + tt * 128: t0 + (tt + 1) * 128,
                                      ot * OT:(ot + 1) * OT],
                                o_s[:])
                    t0 += T

            if repeat > 1:
                with tc.For_i(0, repeat, 1):
                    body()
            else:
                body()

    nc.compile()
    _nc_cache[key] = (nc, out_d.name)
    return nc, out_d.name


def _pack_weights(w1, b1, w2, H, FH, O):
    """Pack per-core weight operands (cached on input array identity)."""
    key = (id(w1), id(w2), id(b1))
    if key in _pack_cache:
        return _pack_cache[key]
    KH = H // 128
    KF = FH // 128
    OT = O // O_TILES
    E = w1.shape[0]
    packs = []
    for e in range(E):
        w1e = np.asarray(w1[e]).astype(BF16)      # [H, 2FH]
        w2e = np.asarray(w2[e]).astype(BF16)      # [2FH, O]
        b1e = np.asarray(b1[e], dtype=np.float32)  # [2FH]
        for half in range(2):
            w1c = w1e[:, half * FH:(half + 1) * FH]
            w2c = w2e[half * FH:(half + 1) * FH, :]
            b1c = b1e[half * FH:(half + 1) * FH]
            w1p = np.ascontiguousarray(
                w1c.reshape(KH, 128, KF, 128).transpose(2, 1, 0, 3))
            w2p = np.ascontiguousarray(
                w2c.reshape(KF, 128, O_TILES, OT).transpose(2, 1, 0, 3))
            b1p = np.ascontiguousarray(b1c.reshape(KF, 128).T)
            packs.append((w1p, w2p, b1p))
    _pack_cache.clear()
    _pack_cache[key] = packs
    return packs


def kernel(hidden_states, router_w, router_b, w1, b1, w2, b2):
    global last_results
    hs = np.asarray(hidden_states, dtype=np.float32)
    rw = np.asarray(router_w, dtype=np.float32)
    rb = np.asarray(router_b, dtype=np.float32)
    w1 = np.asarray(w1)
    w2 = np.asarray(w2)
    b1 = np.asarray(b1)
    b2 = np.asarray(b2, dtype=np.float32)

    B, H = hs.shape
    E = rw.shape[1]
    F2 = w1.shape[2]
    FH = F2 // 2
    O = w2.shape[2]
    assert E * 2 == N_CORES

    # ---- host routing ----
    logits = hs @ rw + rb                       # [B, E] fp32
    m = logits.max(axis=-1, keepdims=True)
    ex = np.exp(logits - m)
    probs = ex / ex.sum(axis=-1, keepdims=True)
    idx = np.argsort(-probs, axis=-1, kind="stable")[:, :TOP_K]   # [B, K]
    p_top = np.take_along_axis(probs, idx, axis=-1)
    gates = (p_top / p_top.sum(axis=-1, keepdims=True)).astype(np.float32)
    gmat = np.zeros((B, E), np.float32)
    np.put_along_axis(gmat, idx, gates, axis=-1)

    sel = np.zeros((B, E), bool)
    np.put_along_axis(sel, idx, True, axis=-1)
    tok = [np.nonzero(sel[:, e])[0] for e in range(E)]
    counts = [len(t) for t in tok]

    # capacity (multiple of 128, same for all cores — SPMD single program)
    C = max(256, int(-(-max(counts) // 128)) * 128)
    KH = H // 128
    n_t = C // 128

    repeat = int(os.environ.get("BASS_KERNEL_REPEAT", "1"))
    nc, out_name = _build(C, H, FH, O, repeat=repeat)
    packs = _pack_weights(w1, b1, w2, H, FH, O)

    # ---- per-core inputs ----
    in_maps = []
    xts = {}
    gps = {}
    for e in range(E):
        n_e = counts[e]
        Xe = np.zeros((C, H), np.float32)
        Xe[:n_e] = hs[tok[e]]
        xt = np.ascontiguousarray(
            Xe.T.astype(BF16).reshape(KH, 128, C).transpose(1, 0, 2))
        xts[e] = xt
        gp = np.zeros((C,), np.float32)
        gp[:n_e] = gmat[tok[e], e]
        gps[e] = np.ascontiguousarray(gp.reshape(n_t, 128).T)
    for c in range(N_CORES):
        e, half = c // 2, c % 2
        w1p, w2p, b1p = packs[c]
        in_maps.append({
            "xt": xts[e], "w1": w1p, "w2": w2p, "b1p": b1p, "gatep": gps[e],
        })

    res = run_bass_kernel_spmd(nc, in_maps, core_ids=list(range(N_CORES)))
    last_results = res

    # ---- host combine ----
    out = gmat @ b2.astype(np.float32)           # [B, O]; zeros in practice
    for e in range(E):
        Y = res.results[2 * e][out_name] + res.results[2 * e + 1][out_name]
        out[tok[e]] += Y[:counts[e]]

    if O == 1792:  # ACTION_DIM=7, NUM_BINS=256
        return out.reshape(B, 7, 256)
    return out.reshape(B, O)
